# revision 37
# baseline (speedup 1.0000x reference)
# Cross-attention kernel for Trainium2 (Bass/Tile), 8-core data-parallel.
#
# Reference computation (per batch element, B=8 -> one batch element per core):
#   q = x1 @ Wq.T + bq ; k = x2 @ Wk.T + bk ; v = x3 @ Wv.T + bv
#   out = softmax(q @ k.T) @ v          (no 1/sqrt(d) scale)
#
# Precision:
#   - q/k projections and q@k.T run in fp32r (fp32 rounded to 11 explicit
#     mantissa bits; full bf16 PE rate when the moving free dim >= 256).
#   - v projection and attn@v run in fp16.
#   - softmax is fp32 (row max on DVE, exp on ScalarE with accumulate,
#     normalization deferred to the output scale).
#
# Schedule (PE roofline: 7 passes of S*C*C MACs ~= 382us warm):
#   - x1/x2/Wq/Wk are transposed ON THE PE (transpose-mode matmul vs a
#     resident fp32 identity, 2 cyc/row), evacuated from PSUM by ScalarE
#     copies that round to fp32r. This kills the v1 hi/lo bf16 split
#     (copy+sub+2 xbar transposes+recombine) whose 6-op chains serialized
#     across engine FIFOs and starved the PE for ~450us.
#   - x3/Wv take the cheap path: SWDGE cast-load (fp32->f16 during DMA)
#     then one xbar DMA transpose; no compute-engine stage at all.
#   - kT [d,k] fp32r (64KiB/part) and v [s,d] f16 (32KiB/part) stay SBUF
#     resident; only qT spills to a DRAM scratch and streams back during
#     attention (8MiB round trip vs 32MiB in v1).
#   - Engine dedication: Pool issues all SWDGE loads, SP all transposes and
#     stores, ACT all PSUM evacs + exp, DVE bias adds + softmax stats. PE
#     FIFO interleaves [T chunk i+2][matmuls chunk i] so transposes hide
#     under matmuls and the PE never waits on prep.
#   - Attention is the v1 structure: per 128-row q tile, scores (c-outer,
#     4 PSUM banks), fp32 softmax, xbar transpose of P to [k,q] f16, then
#     attn@v software-pipelined one tile behind scores.

from contextlib import ExitStack

import numpy as np

import concourse.bass as bass
import concourse.mybir as mybir
import concourse.tile as tile
from concourse import bacc
from concourse.bass_utils import run_bass_kernel_spmd
from concourse.masks import make_identity

F32 = mybir.dt.float32
F32R = mybir.dt.float32r
F16 = mybir.dt.float16
ADD = mybir.AluOpType.add
AX = mybir.AxisListType.X
EXP = mybir.ActivationFunctionType.Exp

B, S, C = 8, 2048, 1024
P = 128
NT_S = S // P  # 16 s-tiles
NT_C = C // P  # 8 c/d-tiles
CH = 512  # free-dim chunk (one fp32 PSUM bank; fp32r full rate needs >=256)
NCH_S = S // CH  # 4
NCH_C = C // CH  # 2
NJ = CH // P  # 4 s-tiles per chunk


def _emit(tc):
    nc = tc.nc

    x1 = nc.dram_tensor("x1", [S, C], F32, kind="ExternalInput").ap()
    x2 = nc.dram_tensor("x2", [S, C], F32, kind="ExternalInput").ap()
    x3 = nc.dram_tensor("x3", [S, C], F32, kind="ExternalInput").ap()
    Wq = nc.dram_tensor("Wq", [C, C], F32, kind="ExternalInput").ap()
    Wk = nc.dram_tensor("Wk", [C, C], F32, kind="ExternalInput").ap()
    Wv = nc.dram_tensor("Wv", [C, C], F32, kind="ExternalInput").ap()
    bq = nc.dram_tensor("bq", [C], F32, kind="ExternalInput").ap()
    bk = nc.dram_tensor("bk", [C], F32, kind="ExternalInput").ap()
    bv = nc.dram_tensor("bv", [C], F32, kind="ExternalInput").ap()
    out = nc.dram_tensor("out", [S, C], F32, kind="ExternalOutput").ap()

    es = ExitStack()
    with es:
        const = es.enter_context(tc.tile_pool(name="const", bufs=1))
        dram = es.enter_context(tc.tile_pool(name="dram", bufs=1, space="DRAM"))

        ident = const.tile([P, P], F32, tag="ident")
        make_identity(nc, ident)

        bq_sb = const.tile([P, NT_C], F32, tag="bq")
        nc.gpsimd.dma_start(out=bq_sb, in_=bq.rearrange("(t p) -> p t", p=P))
        bk_sb = const.tile([P, NT_C], F32, tag="bk")
        nc.gpsimd.dma_start(out=bk_sb, in_=bk.rearrange("(t p) -> p t", p=P))
        bv_sb = const.tile([P, C], F32, tag="bv")

        # DRAM scratch for spilled qT (fp32r bits)
        qT_d = dram.tile([NT_C, P, S], F32R, tag="qTd", name="qTd")

        # kT stays resident through attention: [d_part, dt, k] fp32r
        res_k = es.enter_context(tc.tile_pool(name="resk", bufs=1))
        kT = res_k.tile([P, NT_C, S], F32R, tag="kT", name="kT")

        # WvT resident until end of V phase; prep emitted after the Q phase
        wv_pool = es.enter_context(tc.tile_pool(name="wv", bufs=1))
        WvT = wv_pool.tile([P, NT_C, C], F16, tag="WvT", name="WvT")
        wvload = es.enter_context(tc.tile_pool(name="wvload", bufs=1))

        # NOTE: SWDGE cast-during-DMA (fp32->f16) returned garbage on real
        # hardware when mixed with the rest of the kernel's DMA traffic
        # (isolated micro-tests passed). Use plain loads + ScalarE f16 copy.

        # x3 prep pools live at es level so the first chunk's loads can
        # prefetch during the K phase (no slab-WAR on scope-1 pools)
        x3l = es.enter_context(tc.tile_pool(name="x3l", bufs=2))
        x3t = es.enter_context(tc.tile_pool(name="x3t", bufs=2))

        CHV = 256  # v-phase chunk (smaller: frees SBUF for the stage pool)
        NJV = CHV // P  # 2
        NCHV = S // CHV  # 8

        def prep_x3(ich):
            xT = x3t.tile([P, NT_C, CHV], F16, tag="x3T", name="x3T")
            for j in range(NJV):
                r0 = ich * CHV + j * P
                xsl = x3l.tile([P, C], F32, tag="x3l32", name="x3sl")
                nc.gpsimd.dma_start(out=xsl, in_=x3[r0 : r0 + P, :])
                x16 = x3l.tile([P, C], F16, tag="x3l16", name="x3s16")
                nc.scalar.copy(out=x16, in_=xsl)
                nc.sync.dma_start(
                    out=xT[:, :, j * P : (j + 1) * P], in_=x16,
                    transpose=True,
                )
            return xT

        if True:
            # ---- scope 1: Q then K projections (PE-transpose prep) --------
            with (
                tc.tile_pool(name="xload", bufs=4) as xload,
                tc.tile_pool(name="xt", bufs=2) as xt,
                tc.tile_pool(name="stage", bufs=8) as stage,
                tc.tile_pool(name="tp", bufs=4, space="PSUM") as tp,
                tc.tile_pool(name="mmps", bufs=2, space="PSUM") as mmps,
            ):

                def w_tile(W, wpool, dt):
                    """One W^T tile [128c, ct, 128d] fp32r via PE transpose."""
                    wt = wpool.tile(
                        [P, NT_C, P], F32R, tag=f"W{dt}", name=f"W{dt}"
                    )
                    wsl = xload.tile([P, C], F32, tag="xl", name="wsl")
                    nc.scalar.dma_start(
                        out=wsl, in_=W[dt * P : (dt + 1) * P, :]
                    )
                    for h in range(2):
                        pt = tp.tile([P, CH], F32, tag="pt", name="wpt")
                        for j in range(4):
                            ct = h * 4 + j
                            nc.tensor.transpose(
                                pt[:, j * P : (j + 1) * P],
                                wsl[:, ct * P : (ct + 1) * P],
                                ident,
                            )
                        nc.scalar.copy(
                            out=wt[:, h * 4 : h * 4 + 4, :], in_=pt
                        )
                    return wt

                def prep_x_pe(x, ich):
                    """x^T chunk [128c, ct, CH s] fp32r via PE transpose."""
                    xT = xt.tile([P, NT_C, CH], F32R, tag="xT", name="xT")
                    for j in range(NJ):
                        r0 = ich * CH + j * P
                        xsl = xload.tile([P, C], F32, tag="xl", name="xsl")
                        nc.gpsimd.dma_start(out=xsl, in_=x[r0 : r0 + P, :])
                        for h in range(2):
                            pt = tp.tile([P, CH], F32, tag="pt", name="xpt")
                            for jj in range(4):
                                ct = h * 4 + jj
                                nc.tensor.transpose(
                                    pt[:, jj * P : (jj + 1) * P],
                                    xsl[:, ct * P : (ct + 1) * P],
                                    ident,
                                )
                            nc.scalar.copy(
                                out=xT[:, h * 4 : h * 4 + 4, j * P : (j + 1) * P],
                                in_=pt,
                            )
                    return xT

                def q_dt(xT, ich, wt, dt):
                    """One dt group of a qT chunk -> bias -> spill."""
                    s0 = ich * CH
                    ps = mmps.tile([P, CH], F32, tag="mm", name="qps")
                    for ct in range(NT_C):
                        nc.tensor.matmul(
                            ps,
                            wt[:, ct, :],
                            xT[:, ct, :],
                            start=(ct == 0),
                            stop=(ct == NT_C - 1),
                        )
                    t = stage.tile([P, CH], F32R, tag="qt", name="qt")
                    nc.vector.tensor_scalar_add(
                        out=t, in0=ps, scalar1=bq_sb[:, dt : dt + 1]
                    )
                    # scalar (ACT HWDGE ring): fp32r stores must NOT share
                    # the SP ring with f16 DMA-transposes -- on HW the
                    # fp32r convert state bleeds into concurrent xbar
                    # transposes, rounding the low f16 of each 32b pair.
                    nc.scalar.dma_start(out=qT_d[dt, :, s0 : s0 + CH], in_=t)

                def k_dt(xT, ich, wt, dt):
                    """One dt group of a kT chunk -> bias -> resident kT."""
                    s0 = ich * CH
                    ps = mmps.tile([P, CH], F32, tag="mm", name="kps")
                    for ct in range(NT_C):
                        nc.tensor.matmul(
                            ps,
                            wt[:, ct, :],
                            xT[:, ct, :],
                            start=(ct == 0),
                            stop=(ct == NT_C - 1),
                        )
                    nc.vector.tensor_scalar_add(
                        out=kT[:, dt, s0 : s0 + CH],
                        in0=ps,
                        scalar1=bk_sb[:, dt : dt + 1],
                    )

                def proj_phase(W, wpool, x, dt_fn, mid_hook=None):
                    """W prep interleaved with chunk 0 (paced by W-load
                    arrivals), then chunks 1..3 pipelined on xt bufs=2."""
                    WT = [None] * NT_C
                    WT[0] = w_tile(W, wpool, 0)
                    xT_cur = prep_x_pe(x, 0)
                    WT[1] = w_tile(W, wpool, 1)
                    xT_nxt = None
                    for dt in range(NT_C):
                        if dt + 2 < NT_C:
                            WT[dt + 2] = w_tile(W, wpool, dt + 2)
                        dt_fn(xT_cur, 0, WT[dt], dt)
                        if dt == 3:
                            # c1 prep rides the PE FIFO mid-c0; its loads
                            # are already in flight on the Pool queue
                            xT_nxt = prep_x_pe(x, 1)
                    for ich in range(1, NCH_S):
                        xT_cur = xT_nxt
                        xT_nxt = (
                            prep_x_pe(x, ich + 1) if ich + 1 < NCH_S else None
                        )
                        if mid_hook is not None and ich == 2:
                            mid_hook()
                        for dt in range(NT_C):
                            dt_fn(xT_cur, ich, WT[dt], dt)
                    return WT

                with tc.tile_pool(name="wq", bufs=1) as wq_pool:
                    proj_phase(Wq, wq_pool, x1, q_dt)

                # Wv prep emitted here: loads land on the Pool FIFO after
                # x1's, DMA-transposes on SP after the Wq loads; all of it
                # executes during the Q matmuls.
                for dt in range(NT_C):
                    wsl = xload.tile([P, C], F32, tag="xl", name="wvsl")
                    nc.gpsimd.dma_start(
                        out=wsl, in_=Wv[dt * P : (dt + 1) * P, :]
                    )
                    w16 = wvload.tile([P, C], F16, tag="wvl", name="wvl")
                    nc.vector.tensor_copy(out=w16, in_=wsl)
                    nc.sync.dma_start(
                        out=WvT[:, :, dt * P : (dt + 1) * P], in_=w16,
                        transpose=True,
                    )
                bv_bcast = bass.AP(
                    tensor=bv.tensor, offset=bv.offset, ap=[[0, P], [1, C]]
                )
                nc.gpsimd.dma_start(out=bv_sb, in_=bv_bcast)

                x3_hold = []

                with tc.tile_pool(name="wk", bufs=1) as wk_pool:
                    # x3 chunk-0 prefetch overlaps the K tail
                    proj_phase(
                        Wk, wk_pool, x2, k_dt,
                        mid_hook=lambda: x3_hold.append(prep_x3(0)),
                    )
                x3T_first = x3_hold[0]

            # ---- V phase: cast-load + xbar transpose prep, v resident -----
            res_v = es.enter_context(tc.tile_pool(name="resv", bufs=1))
            v_r = res_v.tile([P, NT_S, C], F16, tag="v", name="v")

            with (
                tc.tile_pool(name="vps", bufs=2, space="PSUM") as vps,
            ):

                def v_chunk(xT, ich):
                    for j in range(NJV):
                        st = ich * NJV + j
                        for cch in range(NCH_C):
                            ps = vps.tile([P, CH], F32, tag="vps", name="vps")
                            for ct in range(NT_C):
                                nc.tensor.matmul(
                                    ps,
                                    xT[:, ct, j * P : (j + 1) * P],
                                    WvT[:, ct, cch * CH : (cch + 1) * CH],
                                    start=(ct == 0),
                                    stop=(ct == NT_C - 1),
                                )
                            nc.vector.tensor_tensor(
                                out=v_r[:, st, cch * CH : (cch + 1) * CH],
                                in0=ps,
                                in1=bv_sb[:, cch * CH : (cch + 1) * CH],
                                op=ADD,
                            )

                xT_cur = x3T_first
                for ich in range(NCHV):
                    nxt = prep_x3(ich + 1) if ich + 1 < NCHV else None
                    v_chunk(xT_cur, ich)
                    xT_cur = nxt

            # ---- attention (kT, v resident; qT streams from DRAM) ---------
            with (
                tc.tile_pool(name="qstream", bufs=2) as qstream,
                tc.tile_pool(name="spsum", bufs=6, space="PSUM") as spsum,
                tc.tile_pool(name="opsum", bufs=2, space="PSUM") as opsum,
                tc.tile_pool(name="attn", bufs=2) as attn,
                tc.tile_pool(name="stats", bufs=4) as stats,
            ):

                def load_q(sq):
                    t = qstream.tile([P, NT_C, P], F32R, tag="qs", name=f"qs{sq}")
                    nc.gpsimd.dma_start(
                        out=t,
                        in_=qT_d[:, :, sq * P : (sq + 1) * P].rearrange(
                            "t p s -> p t s"
                        ),
                    )
                    return t

                def emit_attnv(pT, rinv, sq):
                    ps_o = [
                        opsum.tile([P, CH], F32, tag="o", name=f"o{sq}_{c}")
                        for c in range(NCH_C)
                    ]
                    for skt in range(NT_S):
                        for cch in range(NCH_C):
                            nc.tensor.matmul(
                                ps_o[cch],
                                pT[:, skt, :],
                                v_r[:, skt, cch * CH : (cch + 1) * CH],
                                start=(skt == 0),
                                stop=(skt == NT_S - 1),
                            )
                    o_sb = attn.tile([P, C], F32, tag="osb", name="osb")
                    for cch in range(NCH_C):
                        nc.vector.tensor_scalar_mul(
                            out=o_sb[:, cch * CH : (cch + 1) * CH],
                            in0=ps_o[cch],
                            scalar1=rinv,
                        )
                    nc.scalar.dma_start(out=out[sq * P : (sq + 1) * P, :], in_=o_sb)

                q_cur = load_q(0)
                prev = None
                for sq in range(NT_S):
                    q_next = load_q(sq + 1) if sq + 1 < NT_S else None

                    # scores, c-outer: banks free progressively for sq+1
                    ps_s = [
                        spsum.tile([P, CH], F32, tag="s", name=f"s{sq}_{c}")
                        for c in range(NCH_S)
                    ]
                    for c in range(NCH_S):
                        for dt in range(NT_C):
                            nc.tensor.matmul(
                                ps_s[c],
                                q_cur[:, dt, :],
                                kT[:, dt, c * CH : (c + 1) * CH],
                                start=(dt == 0),
                                stop=(dt == NT_C - 1),
                            )

                    # softmax (fp32, row-wise over the free dim)
                    mx = stats.tile([P, NCH_S], F32, tag="mx", name="mx")
                    for c in range(NCH_S):
                        nc.vector.reduce_max(
                            out=mx[:, c : c + 1], in_=ps_s[c], axis=AX
                        )
                    negmax = stats.tile([P, 1], F32, tag="negmax", name="negmax")
                    nc.vector.reduce_max(out=negmax, in_=mx, axis=AX, negate=True)

                    p_sb = attn.tile([P, S], F16, tag="p", name="p")
                    sums = stats.tile([P, NCH_S], F32, tag="sums", name="sums")
                    for c in range(NCH_S):
                        nc.scalar.activation(
                            out=p_sb[:, c * CH : (c + 1) * CH],
                            in_=ps_s[c],
                            func=EXP,
                            bias=negmax,
                            scale=1.0,
                            accum_out=sums[:, c : c + 1],
                        )
                    rs = stats.tile([P, 1], F32, tag="rs", name="rs")
                    nc.vector.reduce_sum(out=rs, in_=sums, axis=AX)
                    rinv = stats.tile([P, 1], F32, tag="rinv", name="rinv")
                    nc.vector.reciprocal(out=rinv, in_=rs)

                    pT = attn.tile([P, NT_S, P], F16, tag="pT", name="pT")
                    nc.sync.dma_start(out=pT, in_=p_sb, transpose=True)

                    if prev is not None:
                        emit_attnv(*prev)
                    prev = (pT, rinv, sq)
                    q_cur = q_next
                emit_attnv(*prev)


_BUILT = {}


def _build():
    if "nc" not in _BUILT:
        nc = bacc.Bacc(
            "TRN2",
            target_bir_lowering=False,
            debug=False,
            num_devices=B,
        )
        with tile.TileContext(nc) as tc:
            _emit(tc)
        nc.compile()
        _BUILT["nc"] = nc
    return _BUILT["nc"]


def kernel_with_results(trace=False, **inputs):
    nc = _build()
    in_maps = []
    for i in range(B):
        in_maps.append(
            {
                "x1": np.ascontiguousarray(inputs["x1"][i], dtype=np.float32),
                "x2": np.ascontiguousarray(inputs["x2"][i], dtype=np.float32),
                "x3": np.ascontiguousarray(inputs["x3"][i], dtype=np.float32),
                "Wq": np.ascontiguousarray(inputs["Wq"], dtype=np.float32),
                "Wk": np.ascontiguousarray(inputs["Wk"], dtype=np.float32),
                "Wv": np.ascontiguousarray(inputs["Wv"], dtype=np.float32),
                "bq": np.ascontiguousarray(inputs["bq"], dtype=np.float32),
                "bk": np.ascontiguousarray(inputs["bk"], dtype=np.float32),
                "bv": np.ascontiguousarray(inputs["bv"], dtype=np.float32),
            }
        )
    res = run_bass_kernel_spmd(nc, in_maps, core_ids=list(range(B)), trace=trace)
    outs = np.stack([r["out"] for r in res.results], axis=0).astype(np.float32)
    return outs, res


def kernel(**inputs):
    outs, _ = kernel_with_results(trace=False, **inputs)
    return outs
